# revision 7
# baseline (speedup 1.0000x reference)
"""Trainium2 Bass kernel for PVT-style spatial-reduction attention.

Reference computation (per batch):
  q = x @ q_w + q_b                               [4096, 320], 5 heads x 64
  x_ = conv2x2_stride2(x as [64,64,320], sr_w) + sr_b   -> [1024, 320]
  x_ = LayerNorm(x_) * ln_g + ln_b
  kv = x_ @ kv_w + kv_b -> k, v  [1024, 320] each
  out = softmax(q k^T / 8) v  per head -> [4096, 320]
  out = out @ proj_w + proj_b

Sharding: pure data parallelism, batch i -> core i (B == n_cores == 8).
No collectives.

Per-core layout strategy ("T-layout", channels on partitions, tokens on
free dim):
  - x is loaded naturally and PE-transposed into xP: a patch-major view
    x^T[c, token] with tokens grouped by conv-patch offset block
    b = di*2+dj, so both the q projection and the conv reduce to plain
    matmuls over xP.  The token permutation is undone by the final
    strided DMA of the proj output.
  - all matmuls run in float32r (fp32 storage, 1 cyc/row on the PE).
  - biases are folded in via an appended ones-row (K+1 trick); LayerNorm
    gamma/beta are folded into kv_w/kv_b on the host.
  - attention: scores are computed transposed (s^T[m, n]) so softmax's
    sum over keys m is done by the PE itself: v is augmented with a ones
    column, making row 64 of the PV accumulator the softmax denominator.
    exp() runs on ACT fused with the 1/8 scale, psum->sbuf.
    The division is deferred to after PV: 1/sumexp is broadcast over the
    64 head dims with a K=1 ones matmul and applied with one DVE multiply.
"""

import numpy as np

import concourse.bass as bass
import concourse.mybir as mybir
from concourse import bacc
from concourse.tile import TileContext

F32 = mybir.dt.float32
F32R = mybir.dt.float32r
AF = mybir.ActivationFunctionType
ALU = mybir.AluOpType
AX = mybir.AxisListType

B, N, C = 8, 4096, 320
HEAD, HD, SR = 5, 64, 2
NKV = 1024  # (64/2)*(64/2)
SCALE = HD ** -0.5
EPS = 1e-5
N_CORES = 8

# channel chunking of C=320 into partitions: 128 + 128 + 64 (+1 ones row)
CK = [(0, 128), (128, 128), (256, 64)]


def _csz(k, aug):
    # chunk partition count; last chunk optionally includes the ones/bias row
    return CK[k][1] + (1 if (aug and k == 2) else 0)


def build_nc():
    nc = bacc.Bacc()

    x = nc.declare_dram_parameter("x", [N, C], F32, isOutput=False)
    qw = nc.declare_dram_parameter("qw", [C + 1, C], F32, isOutput=False)
    srw = nc.declare_dram_parameter("srw", [4 * C + 1, C], F32, isOutput=False)
    kw = nc.declare_dram_parameter("kw", [C + 1, C], F32, isOutput=False)
    vw = nc.declare_dram_parameter("vw", [C + 1, C], F32, isOutput=False)
    pw = nc.declare_dram_parameter("pw", [C + 1, C], F32, isOutput=False)
    ident = nc.declare_dram_parameter("ident", [128, 128], F32, isOutput=False)
    ones_c = nc.declare_dram_parameter("ones_c", [1, N], F32, isOutput=False)
    ones2 = nc.declare_dram_parameter("ones2", [128, 64], F32, isOutput=False)
    out = nc.declare_dram_parameter("out", [N, C], F32, isOutput=True)

    with TileContext(nc) as tc:
        with (
            nc.allow_low_precision(reason="float32r tiles are full fp32 storage"),
            tc.tile_pool(name="const", bufs=1) as cpool,
            tc.tile_pool(name="main", bufs=1) as mpool,
            tc.tile_pool(name="ps", space="PSUM", bufs=1) as pspool,
        ):
            # ---- constants / weights -------------------------------------
            id_sb = cpool.tile([128, 128], F32R)
            nc.sync.dma_start(out=id_sb[:], in_=ident[:].bitcast(F32R))

            def load_w_chunks(name, dram):
                tiles = []
                for k in range(3):
                    r0 = CK[k][0]
                    p = _csz(k, True)  # last chunk carries the bias row
                    t = cpool.tile([p, C], F32R, name=f"{name}{k}")
                    nc.sync.dma_start(out=t[:], in_=dram[r0 : r0 + p, :].bitcast(F32R))
                    tiles.append(t)
                return tiles

            qw_sb = load_w_chunks("qw_sb", qw)
            kw_sb = load_w_chunks("kw_sb", kw)
            vw_sb = load_w_chunks("vw_sb", vw)
            pw_sb = load_w_chunks("pw_sb", pw)

            # srw rows: [block0 (320), sr_b (1), block1, block2, block3]
            srw_sb = []
            for b in range(4):
                base = 0 if b == 0 else 321 + (b - 1) * C
                tiles = []
                for k in range(3):
                    p = _csz(k, b == 0)  # block0's last chunk has the bias row
                    t = cpool.tile([p, C], F32R, name=f"srw_sb{b}{k}")
                    nc.sync.dma_start(
                        out=t[:], in_=srw[base + CK[k][0] : base + CK[k][0] + p, :].bitcast(F32R)
                    )
                    tiles.append(t)
                srw_sb.append(tiles)

            ones64 = cpool.tile([1, 64], F32R)
            nc.sync.dma_start(out=ones64[:], in_=ones_c[:, 0:64].bitcast(F32R))
            eps_sb = cpool.tile([128, 1], F32)
            nc.vector.memset(eps_sb[:], EPS)

            # ---- long-lived activations ----------------------------------
            # q^T [320, 4096] (token order: xP block-major permutation)
            qT = [
                mpool.tile([_csz(k, False), N], F32R, name=f"qT{k}") for k in range(3)
            ]
            # x_n^T [320(+ones), 1024]
            xnT = [
                mpool.tile([_csz(k, True), NKV], F32R, name=f"xnT{k}") for k in range(3)
            ]
            nc.sync.dma_start(out=xnT[2][64:65, :], in_=ones_c[:, 0:NKV].bitcast(F32R))
            # k^T [320, 1024]
            kT = [
                mpool.tile([_csz(k, False), NKV], F32R, name=f"kT{k}") for k in range(3)
            ]
            # v' [1024, 5*65] stored as [128, 8*325]; col t*325 + h*65 + d,
            # d==64 is the ones column (softmax denominator trick)
            v_all = mpool.tile([128, 8 * 325], F32R)
            nc.sync.dma_start(
                out=v_all.rearrange("p (t h e) -> p t h e", t=8, h=5)[:, :, :, 64],
                in_=ones2[:, 0:40].rearrange("p (t h) -> p t h", t=8).bitcast(F32R),
            )

            # ================= phase A: load x, transpose into xP =========
            # xP[b][k]: [csz, 1024] = x^T[chunk k channels, tokens of patch
            # offset block b], token order p = i*32 + j (conv output order).
            # xP[b][2] has 65 rows, row 64 = ones (bias row for q/conv).
            with tc.tile_pool(name="px", bufs=1) as xpool:
                xP = []
                for b in range(4):
                    row = []
                    for k in range(3):
                        t = xpool.tile([_csz(k, True), NKV], F32R, name=f"xP{b}{k}")
                        row.append(t)
                    nc.sync.dma_start(
                        out=row[2][64:65, :], in_=ones_c[:, 0:NKV].bitcast(F32R)
                    )
                    xP.append(row)

                for g in range(8):  # 512-token windows
                    xa = []
                    for ii in range(4):
                        i = g * 4 + ii
                        t = xpool.tile([128, C], F32R, name="xa", tag="xa", bufs=4)
                        nc.sync.dma_start(
                            out=t[:], in_=x[i * 128 : (i + 1) * 128, :].bitcast(F32R)
                        )
                        xa.append(t)
                    for k in range(3):
                        csz = _csz(k, False)
                        psA = pspool.tile(
                            [csz, 512], F32R, name="psA", tag="ps512", bufs=3
                        )
                        for ii in range(4):
                            nc.tensor.transpose(
                                psA[:, ii * 128 : (ii + 1) * 128],
                                xa[ii][:, CK[k][0] : CK[k][0] + csz],
                                id_sb[:],
                            )
                        # scatter the 512-token window into the 4 patch blocks
                        src = psA.rearrange("p (i a j c) -> p i a j c", i=4, a=2, j=32)
                        for b in range(4):
                            di, dj = b // 2, b % 2
                            dst = xP[b][k][0:csz, g * 128 : (g + 1) * 128].rearrange(
                                "p (i j) -> p i j", i=4
                            )
                            nc.vector.tensor_copy(dst, src[:, :, di, :, dj])

                # ============= phase B: q^T = qw^T @ x'^T =================
                for b in range(4):
                    for m in range(3):
                        msz = _csz(m, False)
                        for nn in range(2):
                            psB = pspool.tile(
                                [msz, 512], F32, name="psB", tag="ps512", bufs=3
                            )
                            for k in range(3):
                                kp = _csz(k, True)
                                nc.tensor.matmul(
                                    psB[:],
                                    qw_sb[k][:, CK[m][0] : CK[m][0] + msz],
                                    xP[b][k][0:kp, nn * 512 : (nn + 1) * 512],
                                    start=(k == 0),
                                    stop=(k == 2),
                                )
                            nc.scalar.copy(
                                qT[m][:, b * 1024 + nn * 512 : b * 1024 + (nn + 1) * 512],
                                psB[:],
                            )

                # ============= phase C: conv + LayerNorm ==================
                for t8 in range(8):
                    psC = pspool.tile([128, C], F32, name="psC", tag="ps320", bufs=2)
                    idx = 0
                    for b in range(4):
                        for k in range(3):
                            kp = _csz(k, b == 0)
                            nc.tensor.matmul(
                                psC[:],
                                xP[b][k][0:kp, t8 * 128 : (t8 + 1) * 128],
                                srw_sb[b][k][0:kp, :],
                                start=(idx == 0),
                                stop=(idx == 11),
                            )
                            idx += 1
                    # LayerNorm over the free dim (channels)
                    sum_t = xpool.tile([128, 1], F32, name="sum_t", tag="st1", bufs=4)
                    negmu = xpool.tile([128, 1], F32, name="negmu", tag="st2", bufs=4)
                    vsum = xpool.tile([128, 1], F32, name="vsum", tag="st3", bufs=4)
                    sd = xpool.tile([128, 1], F32, name="sd", tag="st4", bufs=4)
                    inv = xpool.tile([128, 1], F32, name="inv", tag="st5", bufs=4)
                    scr = xpool.tile([128, C], F32, name="scr", tag="scr", bufs=2)
                    scr2 = xpool.tile([128, C], F32, name="scr2", tag="scr2", bufs=2)
                    xn = xpool.tile([128, C], F32R, name="xn", tag="xn", bufs=3)

                    nc.vector.tensor_reduce(sum_t[:], psC[:], axis=AX.X, op=ALU.add)
                    nc.vector.tensor_scalar_mul(negmu[:], sum_t[:], -1.0 / C)
                    # centered = x - mu  (SBUF), then vsum = sum(centered^2)
                    nc.vector.tensor_scalar(
                        out=scr[:], in0=psC[:], scalar1=negmu[:], scalar2=None, op0=ALU.add
                    )
                    nc.vector.scalar_tensor_tensor(
                        out=scr2[:],
                        in0=scr[:],
                        scalar=0.0,
                        in1=scr[:],
                        op0=ALU.add,
                        op1=ALU.mult,
                        accum_out=vsum[:],
                    )
                    nc.scalar.activation(sd[:], vsum[:], AF.Sqrt, scale=1.0 / C, bias=eps_sb[:])
                    nc.vector.reciprocal(inv[:], sd[:])
                    nc.vector.tensor_scalar_mul(xn[:], scr[:], inv[:])
                    # transpose x_n -> xnT
                    for k in range(3):
                        csz = _csz(k, False)
                        psD = pspool.tile(
                            [csz, 128], F32R, name="psD", tag="ps320", bufs=2
                        )
                        nc.tensor.transpose(
                            psD[:], xn[:, CK[k][0] : CK[k][0] + csz], id_sb[:]
                        )
                        nc.scalar.copy(
                            xnT[k][0:csz, t8 * 128 : (t8 + 1) * 128], psD[:]
                        )

            # ================= phase D: k^T and v' ========================
            for m in range(3):
                msz = _csz(m, False)
                for nn in range(2):
                    psE = pspool.tile([msz, 512], F32, name="psE", tag="ps512", bufs=3)
                    for k in range(3):
                        kp = _csz(k, True)
                        nc.tensor.matmul(
                            psE[:],
                            kw_sb[k][:, CK[m][0] : CK[m][0] + msz],
                            xnT[k][0:kp, nn * 512 : (nn + 1) * 512],
                            start=(k == 0),
                            stop=(k == 2),
                        )
                    nc.scalar.copy(kT[m][:, nn * 512 : (nn + 1) * 512], psE[:])

            for t8 in range(8):
                psF = pspool.tile([128, C], F32, name="psF", tag="ps320", bufs=2)
                for k in range(3):
                    kp = _csz(k, True)
                    nc.tensor.matmul(
                        psF[:],
                        xnT[k][0:kp, t8 * 128 : (t8 + 1) * 128],
                        vw_sb[k][0:kp, :],
                        start=(k == 0),
                        stop=(k == 2),
                    )
                dst = v_all[:, t8 * 325 : (t8 + 1) * 325].rearrange(
                    "p (h e) -> p h e", h=5
                )[:, :, 0:64]
                nc.scalar.copy(dst, psF.rearrange("p (h d) -> p h d", h=5))

            # ================= phase E: attention + proj ==================
            with tc.tile_pool(name="att", bufs=1) as apool:
                attnT = [
                    apool.tile([_csz(k, True), N], F32R, name=f"attnT{k}")
                    for k in range(3)
                ]
                nc.sync.dma_start(
                    out=attnT[2][64:65, :], in_=ones_c[:, 0:N].bitcast(F32R)
                )

                out_r = out.rearrange("(i a j c) d -> i a j c d", a=2, j=32, c=2)

                for n in range(8):  # 512-token column chunks (xP order)
                    for h in range(HEAD):
                        hc, hr = h // 2, (h % 2) * 64
                        psO = pspool.tile([65, 512], F32, name="psO", tag="psacc", bufs=2)
                        p_tiles = []
                        # software-pipelined: scores(mc+1) issued before PV(mc)
                        for mc in range(8):
                            psS = pspool.tile(
                                [128, 512], F32, name="psS", tag="ps512", bufs=3
                            )
                            nc.tensor.matmul(
                                psS[:],
                                kT[hc][hr : hr + 64, mc * 128 : (mc + 1) * 128],
                                qT[hc][hr : hr + 64, n * 512 : (n + 1) * 512],
                                start=True,
                                stop=True,
                            )
                            p_t = apool.tile(
                                [128, 512], F32R, name="p_t", tag="p_t", bufs=4
                            )
                            nc.scalar.activation(p_t[:], psS[:], AF.Exp, scale=SCALE)
                            p_tiles.append(p_t)
                            if mc >= 1:
                                nc.tensor.matmul(
                                    psO[:],
                                    v_all[:, (mc - 1) * 325 + h * 65 : (mc - 1) * 325 + (h + 1) * 65],
                                    p_tiles[mc - 1][:],
                                    start=(mc - 1 == 0),
                                    stop=False,
                                    skip_group_check=True,
                                )
                        nc.tensor.matmul(
                            psO[:],
                            v_all[:, 7 * 325 + h * 65 : 7 * 325 + (h + 1) * 65],
                            p_tiles[7][:],
                            start=False,
                            stop=True,
                            skip_group_check=True,
                        )
                        # divide by sumexp (row 64 of psO)
                        o0 = apool.tile([65, 512], F32, name="o0", tag="o0", bufs=3)
                        nc.vector.tensor_copy(o0[:], psO[:])
                        rec = apool.tile([1, 512], F32R, name="rec", tag="rec", bufs=4)
                        nc.vector.reciprocal(rec[:], o0[64:65, :])
                        psRb = pspool.tile([64, 512], F32, name="psRb", tag="ps512", bufs=3)
                        nc.tensor.matmul(
                            psRb[:], ones64[:], rec[:], start=True, stop=True
                        )
                        nc.vector.tensor_tensor(
                            out=attnT[hc][hr : hr + 64, n * 512 : (n + 1) * 512],
                            in0=o0[0:64, :],
                            in1=psRb[:],
                            op=ALU.mult,
                        )
                    # proj for the 4 128-token chunks of this n-chunk
                    for tt in range(4):
                        t = n * 4 + tt
                        psP = pspool.tile([128, C], F32, name="psP", tag="ps320", bufs=2)
                        for k in range(3):
                            kp = _csz(k, True)
                            nc.tensor.matmul(
                                psP[:],
                                attnT[k][0:kp, t * 128 : (t + 1) * 128],
                                pw_sb[k][0:kp, :],
                                start=(k == 0),
                                stop=(k == 2),
                            )
                        o_sb = apool.tile([128, C], F32, name="o_sb", tag="o_sb", bufs=3)
                        nc.vector.tensor_copy(o_sb[:], psP[:])
                        b, i0 = t // 8, (t % 8) * 4
                        di, dj = b // 2, b % 2
                        nc.sync.dma_start(
                            out=out_r[i0 : i0 + 4, di, :, dj, :], in_=o_sb[:]
                        )

    nc.compile()
    return nc


_CACHE = {}


def _prep_inputs(inputs):
    x = np.ascontiguousarray(np.asarray(inputs["x"], dtype=np.float32))
    q_w = np.asarray(inputs["q_w"], np.float32)
    q_b = np.asarray(inputs["q_b"], np.float32)
    kv_w = np.asarray(inputs["kv_w"], np.float32)
    kv_b = np.asarray(inputs["kv_b"], np.float32)
    sr_w = np.asarray(inputs["sr_w"], np.float32)
    sr_b = np.asarray(inputs["sr_b"], np.float32)
    ln_g = np.asarray(inputs["ln_g"], np.float32)
    ln_b = np.asarray(inputs["ln_b"], np.float32)
    proj_w = np.asarray(inputs["proj_w"], np.float32)
    proj_b = np.asarray(inputs["proj_b"], np.float32)

    qw = np.concatenate([q_w, q_b[None, :]], axis=0)  # [321, 320]
    blocks = sr_w.reshape(4, C, C)  # HWIO -> (di*2+dj, ci, co)
    srw = np.concatenate(
        [blocks[0], sr_b[None, :], blocks[1], blocks[2], blocks[3]], axis=0
    )  # [1281, 320]
    kv_w_eff = ln_g[:, None] * kv_w
    kv_bias = ln_b @ kv_w + kv_b
    kw = np.concatenate([kv_w_eff[:, :C], kv_bias[None, :C]], axis=0)
    vw = np.concatenate([kv_w_eff[:, C:], kv_bias[None, C:]], axis=0)
    pw = np.concatenate([proj_w, proj_b[None, :]], axis=0)

    consts = {
        "qw": np.ascontiguousarray(qw),
        "srw": np.ascontiguousarray(srw),
        "kw": np.ascontiguousarray(kw),
        "vw": np.ascontiguousarray(vw),
        "pw": np.ascontiguousarray(pw),
        "ident": np.eye(128, dtype=np.float32),
        "ones_c": np.ones((1, N), np.float32),
        "ones2": np.ones((128, 64), np.float32),
    }
    return x, consts


def kernel(**inputs) -> np.ndarray:
    H = int(np.asarray(inputs["H"]))
    W = int(np.asarray(inputs["W"]))
    assert H == 64 and W == 64, (H, W)
    x, consts = _prep_inputs(inputs)
    assert x.shape == (B, N, C)

    if "nc" not in _CACHE:
        _CACHE["nc"] = build_nc()
    nc = _CACHE["nc"]

    from concourse.bass_utils import run_bass_kernel_spmd

    in_maps = [{"x": x[i], **consts} for i in range(N_CORES)]
    res = run_bass_kernel_spmd(nc, in_maps, core_ids=list(range(N_CORES)))
    out = np.stack([res.results[i]["out"] for i in range(N_CORES)], axis=0)
    return out.astype(np.float32)


# revision 18
# speedup vs baseline: 6469.3597x; 6469.3597x over previous
"""Trainium2 Bass kernel for PVT-style spatial-reduction attention.

Reference computation (per batch):
  q = x @ q_w + q_b                               [4096, 320], 5 heads x 64
  x_ = conv2x2_stride2(x as [64,64,320], sr_w) + sr_b   -> [1024, 320]
  x_ = LayerNorm(x_) * ln_g + ln_b
  kv = x_ @ kv_w + kv_b -> k, v  [1024, 320] each
  out = softmax(q k^T / 8) v  per head -> [4096, 320]
  out = out @ proj_w + proj_b

Sharding: pure data parallelism, batch i -> core i (B == n_cores == 8).
No collectives.

Per-core layout strategy ("T-layout", channels on partitions, tokens on
free dim):
  - x is loaded naturally and PE-transposed into xP: a patch-major view
    x^T[c, token] with tokens grouped by conv-patch offset block
    b = di*2+dj, so both the q projection and the conv reduce to plain
    matmuls over xP.  The token permutation is undone by the final
    strided DMA of the proj output.
  - all matmuls run in float32r (fp32 storage, 1 cyc/row on the PE).
  - biases are folded in via an appended ones-row (K+1 trick); LayerNorm
    gamma/beta are folded into kv_w/kv_b on the host.
  - the serial head (x ingest -> conv -> LN -> x_n^T -> k/v) is emitted
    window-by-window so the kv chain completes as early as possible; the
    q projection is interleaved into the attention loop, where ACT (exp)
    is the bottleneck and the PE has slack.
  - attention: scores are computed transposed (s^T[m, n]) so softmax's
    sum over keys m is done by the PE itself: v is augmented with a ones
    column, making row 64 of the PV accumulator the softmax denominator.
    exp() runs on ACT fused with the 1/8 scale, psum->sbuf.
    Division by sumexp happens after PV: reciprocal (DVE) is partition-
    broadcast by the otherwise-idle GPSIMD engine and applied with one
    DVE multiply.  The division chain of pair i is emitted after pair
    i+1's matmuls so the in-order PE stream never stalls on it.
"""

import numpy as np

import concourse.bass as bass
import concourse.mybir as mybir
from concourse import bacc
from concourse.tile import TileContext

F32 = mybir.dt.float32
F32R = mybir.dt.float32r
AF = mybir.ActivationFunctionType
ALU = mybir.AluOpType
AX = mybir.AxisListType

B, N, C = 8, 4096, 320
HEAD, HD, SR = 5, 64, 2
NKV = 1024  # (64/2)*(64/2)
SCALE = HD ** -0.5
EPS = 1e-5
N_CORES = 8

# channel chunking of C=320 into partitions: 128 + 128 + 64 (+1 ones row)
CK = [(0, 128), (128, 128), (256, 64)]


def _csz(k, aug):
    return CK[k][1] + (1 if (aug and k == 2) else 0)


def build_nc(repeat=1):
    nc = bacc.Bacc()

    x = nc.declare_dram_parameter("x", [N, C], F32, isOutput=False)
    qw = nc.declare_dram_parameter("qw", [C + 1, C], F32, isOutput=False)
    srw = nc.declare_dram_parameter("srw", [4 * C + 1, C], F32, isOutput=False)
    kw = nc.declare_dram_parameter("kw", [C + 1, C], F32, isOutput=False)
    vw = nc.declare_dram_parameter("vw", [C + 1, C], F32, isOutput=False)
    pw = nc.declare_dram_parameter("pw", [C + 1, C], F32, isOutput=False)
    ident = nc.declare_dram_parameter("ident", [128, 128], F32, isOutput=False)
    ones_c = nc.declare_dram_parameter("ones_c", [1, N], F32, isOutput=False)
    ones2 = nc.declare_dram_parameter("ones2", [128, 64], F32, isOutput=False)
    out = nc.declare_dram_parameter("out", [N, C], F32, isOutput=True)

    import contextlib

    with TileContext(nc) as tc:
        with (
            nc.allow_low_precision(reason="float32r tiles are full fp32 storage"),
            tc.tile_pool(name="const", bufs=1) as cpool,
            tc.tile_pool(name="main", bufs=1) as mpool,
            tc.tile_pool(name="ps", space="PSUM", bufs=1) as pspool,
            tc.For_i(0, repeat, 1) if repeat > 1 else contextlib.nullcontext(),
        ):
            # ---- tile allocations ----------------------------------------
            id_sb = cpool.tile([128, 128], F32R)
            qw_sb = [
                cpool.tile([_csz(k, True), C], F32R, name=f"qw_sb{k}") for k in range(3)
            ]
            kw_sb = [
                cpool.tile([_csz(k, True), C], F32R, name=f"kw_sb{k}") for k in range(3)
            ]
            vw_sb = [
                cpool.tile([_csz(k, True), C], F32R, name=f"vw_sb{k}") for k in range(3)
            ]
            pw_sb = [
                cpool.tile([_csz(k, True), C], F32R, name=f"pw_sb{k}") for k in range(3)
            ]
            srw_sb = [
                [
                    cpool.tile([_csz(k, b == 0), C], F32R, name=f"srw_sb{b}{k}")
                    for k in range(3)
                ]
                for b in range(4)
            ]
            eps_sb = cpool.tile([128, 1], F32)

            xP = [
                [
                    mpool.tile([_csz(k, True), NKV], F32R, name=f"xP{b}{k}")
                    for k in range(3)
                ]
                for b in range(4)
            ]
            xnT = [
                mpool.tile([_csz(k, True), NKV], F32R, name=f"xnT{k}") for k in range(3)
            ]
            kT = [
                mpool.tile([_csz(k, False), NKV], F32R, name=f"kT{k}") for k in range(3)
            ]
            # v' [1024, 5*65] stored as [128, 8*325]; col t*325 + h*65 + d,
            # d==64 is the ones column (softmax denominator trick)
            v_all = mpool.tile([128, 8 * 325], F32R)

            # ============ head: x ingest + conv + LayerNorm + x_n^T =======
            # software-pipelined by one window: while window g is being
            # transposed/scattered, window g-1 runs conv+LN and window g-2
            # runs its v projection, so no engine waits on the serial chain.
            with tc.tile_pool(name="px", bufs=1) as xpool:

                def emit_window_ingest(g):
                    xa = []
                    for ii in range(4):
                        i = g * 4 + ii
                        t = xpool.tile([128, C], F32R, name="xa", tag="xa", bufs=6)
                        nc.sync.dma_start(
                            out=t[:], in_=x[i * 128 : (i + 1) * 128, :].bitcast(F32R)
                        )
                        xa.append(t)
                    for k in range(3):
                        csz = _csz(k, False)
                        psA = pspool.tile(
                            [csz, 512], F32R, name="psA", tag="ps512", bufs=3
                        )
                        for ii in range(4):
                            nc.tensor.transpose(
                                psA[:, ii * 128 : (ii + 1) * 128],
                                xa[ii][:, CK[k][0] : CK[k][0] + csz],
                                id_sb[:],
                            )
                        # scatter the 512-token window into the 4 patch blocks
                        src = psA.rearrange("p (i a j c) -> p i a j c", i=4, a=2, j=32)
                        for b in range(4):
                            di, dj = b // 2, b % 2
                            dst = xP[b][k][0:csz, g * 128 : (g + 1) * 128].rearrange(
                                "p (i j) -> p i j", i=4
                            )
                            if g >= 4 or b % 2 == 0:
                                nc.vector.tensor_copy(dst, src[:, :, di, :, dj])
                            else:
                                nc.scalar.copy(dst, src[:, :, di, :, dj])

                def emit_conv_ln(t8):
                    psC = pspool.tile([128, C], F32, name="psC", tag="ps320", bufs=2)
                    idx = 0
                    for b in range(4):
                        for k in range(3):
                            kp = _csz(k, b == 0)
                            nc.tensor.matmul(
                                psC[:],
                                xP[b][k][0:kp, t8 * 128 : (t8 + 1) * 128],
                                srw_sb[b][k][0:kp, :],
                                start=(idx == 0),
                                stop=(idx == 11),
                            )
                            idx += 1
                    sum_t = xpool.tile([128, 1], F32, name="sum_t", tag="st1", bufs=4)
                    negmu = xpool.tile([128, 1], F32, name="negmu", tag="st2", bufs=4)
                    vsum = xpool.tile([128, 1], F32, name="vsum", tag="st3", bufs=4)
                    sd = xpool.tile([128, 1], F32, name="sd", tag="st4", bufs=4)
                    inv = xpool.tile([128, 1], F32, name="inv", tag="st5", bufs=4)
                    scr = xpool.tile([128, C], F32, name="scr", tag="scr", bufs=3)
                    scr2 = xpool.tile([128, C], F32, name="scr2", tag="scr2", bufs=2)
                    xn = xpool.tile([128, C], F32R, name="xn", tag="xn", bufs=3)

                    nc.vector.tensor_reduce(sum_t[:], psC[:], axis=AX.X, op=ALU.add)
                    nc.vector.tensor_scalar_mul(negmu[:], sum_t[:], -1.0 / C)
                    # centered = x - mu  (SBUF), then vsum = sum(centered^2)
                    nc.vector.tensor_scalar(
                        out=scr[:], in0=psC[:], scalar1=negmu[:], scalar2=None, op0=ALU.add
                    )
                    nc.vector.scalar_tensor_tensor(
                        out=scr2[:],
                        in0=scr[:],
                        scalar=0.0,
                        in1=scr[:],
                        op0=ALU.add,
                        op1=ALU.mult,
                        accum_out=vsum[:],
                    )
                    nc.scalar.activation(
                        sd[:], vsum[:], AF.Sqrt, scale=1.0 / C, bias=eps_sb[:]
                    )
                    nc.vector.reciprocal(inv[:], sd[:])
                    nc.vector.tensor_scalar_mul(xn[:], scr[:], inv[:])
                    # transpose x_n -> xnT
                    for k in range(3):
                        csz = _csz(k, False)
                        psD = pspool.tile(
                            [csz, 128], F32R, name="psD", tag="ps512", bufs=3
                        )
                        nc.tensor.transpose(
                            psD[:], xn[:, CK[k][0] : CK[k][0] + csz], id_sb[:]
                        )
                        nc.scalar.copy(xnT[k][0:csz, t8 * 128 : (t8 + 1) * 128], psD[:])

                def emit_v(t8):
                    psF = pspool.tile([128, C], F32, name="psF", tag="ps320", bufs=2)
                    for k in range(3):
                        kp = _csz(k, True)
                        nc.tensor.matmul(
                            psF[:],
                            xnT[k][0:kp, t8 * 128 : (t8 + 1) * 128],
                            vw_sb[k][0:kp, :],
                            start=(k == 0),
                            stop=(k == 2),
                        )
                    dst = v_all[:, t8 * 325 : (t8 + 1) * 325].rearrange(
                        "p (h e) -> p h e", h=5
                    )[:, :, 0:64]
                    nc.scalar.copy(dst, psF.rearrange("p (h d) -> p h d", h=5))

                def emit_kT(nn):
                    for m in range(3):
                        msz = _csz(m, False)
                        psE = pspool.tile(
                            [msz, 512], F32, name="psE", tag="ps512", bufs=3
                        )
                        for k in range(3):
                            kp = _csz(k, True)
                            nc.tensor.matmul(
                                psE[:],
                                kw_sb[k][:, CK[m][0] : CK[m][0] + msz],
                                xnT[k][0:kp, nn * 512 : (nn + 1) * 512],
                                start=(k == 0),
                                stop=(k == 2),
                            )
                        nc.scalar.copy(kT[m][:, nn * 512 : (nn + 1) * 512], psE[:])

                # priority order: identity + first x window, then conv
                # weights, then the pipelined window loop
                nc.sync.dma_start(out=id_sb[:], in_=ident[:].bitcast(F32R))
                nc.vector.memset(eps_sb[:], EPS)
                emit_window_ingest(0)
                for b in range(4):
                    base = 0 if b == 0 else 321 + (b - 1) * C
                    for k in range(3):
                        p = _csz(k, b == 0)
                        nc.sync.dma_start(
                            out=srw_sb[b][k][:],
                            in_=srw[base + CK[k][0] : base + CK[k][0] + p, :].bitcast(F32R),
                        )
                for b in range(4):
                    nc.sync.dma_start(
                        out=xP[b][2][64:65, :], in_=ones_c[:, 0:NKV].bitcast(F32R)
                    )
                for k in range(3):
                    r0, p = CK[k][0], _csz(k, True)
                    nc.sync.dma_start(out=vw_sb[k][:], in_=vw[r0 : r0 + p, :].bitcast(F32R))
                    nc.sync.dma_start(out=kw_sb[k][:], in_=kw[r0 : r0 + p, :].bitcast(F32R))
                nc.sync.dma_start(
                    out=xnT[2][64:65, :], in_=ones_c[:, 0:NKV].bitcast(F32R)
                )
                nc.sync.dma_start(
                    out=v_all.rearrange("p (t h e) -> p t h e", t=8, h=5)[:, :, :, 64],
                    in_=ones2[:, 0:40].rearrange("p (t h) -> p t h", t=8).bitcast(F32R),
                )

                for g in range(1, 8):
                    emit_window_ingest(g)
                    emit_conv_ln(g - 1)
                    if g >= 2:
                        emit_v(g - 2)
                    if g == 5:
                        emit_kT(0)
                    if g == 2:
                        for k in range(3):
                            r0, p = CK[k][0], _csz(k, True)
                            nc.sync.dma_start(
                                out=qw_sb[k][:], in_=qw[r0 : r0 + p, :].bitcast(F32R)
                            )
                            nc.sync.dma_start(
                                out=pw_sb[k][:], in_=pw[r0 : r0 + p, :].bitcast(F32R)
                            )
                emit_conv_ln(7)
                emit_v(6)
                emit_v(7)
                emit_kT(1)

            # ================= attention + q-proj + out-proj ==============
            with tc.tile_pool(name="att", bufs=1) as apool:
                attnT = [
                    apool.tile([_csz(k, True), N], F32R, name=f"attnT{k}")
                    for k in range(3)
                ]
                nc.sync.dma_start(
                    out=attnT[2][64:65, :], in_=ones_c[:, 0:N].bitcast(F32R)
                )

                out_r = out.rearrange("(i a j c) d -> i a j c d", a=2, j=32, c=2)

                def emit_q(n):
                    # q^T for 512-token column chunk n -> rotating tiles
                    b, nn = n // 2, n % 2
                    tiles = []
                    for m in range(3):
                        msz = _csz(m, False)
                        psB = pspool.tile(
                            [msz, 512], F32, name="psB", tag="psb", bufs=1
                        )
                        for k in range(3):
                            kp = _csz(k, True)
                            nc.tensor.matmul(
                                psB[:],
                                qw_sb[k][:, CK[m][0] : CK[m][0] + msz],
                                xP[b][k][0:kp, nn * 512 : (nn + 1) * 512],
                                start=(k == 0),
                                stop=(k == 2),
                            )
                        qtn = apool.tile(
                            [msz, 512], F32R, name=f"qTn{m}", tag=f"qtn{m}", bufs=2
                        )
                        nc.vector.tensor_copy(qtn[:], psB[:])
                        tiles.append(qtn)
                    return tiles

                def emit_division(n, h, psO):
                    hc, hr = h // 2, (h % 2) * 64
                    o0 = apool.tile([65, 512], F32, name="o0", tag="o0", bufs=3)
                    nc.vector.tensor_copy(o0[:], psO[:])
                    rec = apool.tile([1, 512], F32R, name="rec", tag="rec", bufs=2)
                    nc.vector.reciprocal(rec[:], o0[64:65, :])
                    rb = apool.tile([64, 512], F32R, name="rb", tag="rb", bufs=2)
                    nc.gpsimd.partition_broadcast(rb[:], rec[:])
                    nc.gpsimd.tensor_tensor(
                        out=attnT[hc][hr : hr + 64, n * 512 : (n + 1) * 512],
                        in0=o0[0:64, :],
                        in1=rb[:],
                        op=ALU.mult,
                    )

                proj_queue = []

                def emit_proj_unit():
                    if not proj_queue:
                        return
                    t = proj_queue.pop(0)
                    psP = pspool.tile([128, C], F32, name="psP", tag="ps320", bufs=2)
                    for k in range(3):
                        kp = _csz(k, True)
                        nc.tensor.matmul(
                            psP[:],
                            attnT[k][0:kp, t * 128 : (t + 1) * 128],
                            pw_sb[k][0:kp, :],
                            start=(k == 0),
                            stop=(k == 2),
                        )
                    o_sb = apool.tile([128, C], F32, name="o_sb", tag="o_sb", bufs=4)
                    nc.vector.tensor_copy(o_sb[:], psP[:])
                    b, i0 = t // 8, (t % 8) * 4
                    di, dj = b // 2, b % 2
                    nc.sync.dma_start(out=out_r[i0 : i0 + 4, di, :, dj, :], in_=o_sb[:])

                # q for chunk 0 before attention starts; later chunks are
                # interleaved (the PE has slack in the ACT-bound pairs)
                qcur = emit_q(0)

                pending = None  # (n, h, psO) whose division is not yet emitted
                for n in range(8):  # 512-token column chunks (xP order)
                    qnext = None
                    for h in range(HEAD):
                        hc, hr = h // 2, (h % 2) * 64
                        psO = pspool.tile([65, 512], F32, name="psO", tag="psacc", bufs=2)
                        p_tiles = []
                        # software-pipelined: scores(mc+1) issued before PV(mc)
                        for mc in range(8):
                            psS = pspool.tile(
                                [128, 512], F32, name="psS", tag="ps512", bufs=3
                            )
                            nc.tensor.matmul(
                                psS[:],
                                kT[hc][hr : hr + 64, mc * 128 : (mc + 1) * 128],
                                qcur[hc][hr : hr + 64, :],
                                start=True,
                                stop=True,
                            )
                            p_t = apool.tile(
                                [128, 512], F32R, name="p_t", tag="p_t", bufs=4
                            )
                            nc.scalar.activation(p_t[:], psS[:], AF.Exp, scale=SCALE)
                            p_tiles.append(p_t)
                            if mc == 4 and pending is not None:
                                emit_division(*pending)
                                if pending[1] == HEAD - 1:
                                    proj_queue.extend(
                                        range(pending[0] * 4, pending[0] * 4 + 4)
                                    )
                                pending = None
                                emit_proj_unit()
                            if mc >= 1:
                                nc.tensor.matmul(
                                    psO[:],
                                    v_all[:, (mc - 1) * 325 + h * 65 : (mc - 1) * 325 + (h + 1) * 65],
                                    p_tiles[mc - 1][:],
                                    start=(mc - 1 == 0),
                                    stop=False,
                                    skip_group_check=True,
                                )
                        nc.tensor.matmul(
                            psO[:],
                            v_all[:, 7 * 325 + h * 65 : 7 * 325 + (h + 1) * 65],
                            p_tiles[7][:],
                            start=False,
                            stop=True,
                            skip_group_check=True,
                        )
                        pending = (n, h, psO)
                        # next chunk's q after the first pair of this chunk
                        if h == 0 and n < 7:
                            qnext = emit_q(n + 1)
                    qcur = qnext
                emit_division(*pending)
                proj_queue.extend(range(pending[0] * 4, pending[0] * 4 + 4))
                while proj_queue:
                    emit_proj_unit()

    nc.compile()
    return nc


_CACHE = {}


def _prep_inputs(inputs):
    x = np.ascontiguousarray(np.asarray(inputs["x"], dtype=np.float32))
    q_w = np.asarray(inputs["q_w"], np.float32)
    q_b = np.asarray(inputs["q_b"], np.float32)
    kv_w = np.asarray(inputs["kv_w"], np.float32)
    kv_b = np.asarray(inputs["kv_b"], np.float32)
    sr_w = np.asarray(inputs["sr_w"], np.float32)
    sr_b = np.asarray(inputs["sr_b"], np.float32)
    ln_g = np.asarray(inputs["ln_g"], np.float32)
    ln_b = np.asarray(inputs["ln_b"], np.float32)
    proj_w = np.asarray(inputs["proj_w"], np.float32)
    proj_b = np.asarray(inputs["proj_b"], np.float32)

    qw = np.concatenate([q_w, q_b[None, :]], axis=0)  # [321, 320]
    blocks = sr_w.reshape(4, C, C)  # HWIO -> (di*2+dj, ci, co)
    srw = np.concatenate(
        [blocks[0], sr_b[None, :], blocks[1], blocks[2], blocks[3]], axis=0
    )  # [1281, 320]
    kv_w_eff = ln_g[:, None] * kv_w
    kv_bias = ln_b @ kv_w + kv_b
    kw = np.concatenate([kv_w_eff[:, :C], kv_bias[None, :C]], axis=0)
    vw = np.concatenate([kv_w_eff[:, C:], kv_bias[None, C:]], axis=0)
    pw = np.concatenate([proj_w, proj_b[None, :]], axis=0)

    consts = {
        "qw": np.ascontiguousarray(qw),
        "srw": np.ascontiguousarray(srw),
        "kw": np.ascontiguousarray(kw),
        "vw": np.ascontiguousarray(vw),
        "pw": np.ascontiguousarray(pw),
        "ident": np.eye(128, dtype=np.float32),
        "ones_c": np.ones((1, N), np.float32),
        "ones2": np.ones((128, 64), np.float32),
    }
    return x, consts


def kernel(**inputs) -> np.ndarray:
    H = int(np.asarray(inputs["H"]))
    W = int(np.asarray(inputs["W"]))
    assert H == 64 and W == 64, (H, W)
    x, consts = _prep_inputs(inputs)
    assert x.shape == (B, N, C)

    if "nc" not in _CACHE:
        _CACHE["nc"] = build_nc()
    nc = _CACHE["nc"]

    from concourse.bass_utils import run_bass_kernel_spmd

    in_maps = [{"x": x[i], **consts} for i in range(N_CORES)]
    res = run_bass_kernel_spmd(nc, in_maps, core_ids=list(range(N_CORES)))
    out = np.stack([res.results[i]["out"] for i in range(N_CORES)], axis=0)
    return out.astype(np.float32)


# revision 38
# speedup vs baseline: 9266.6661x; 1.4324x over previous
"""Trainium2 Bass kernel for PVT-style spatial-reduction attention.

Reference computation (per batch):
  q = x @ q_w + q_b                               [4096, 320], 5 heads x 64
  x_ = conv2x2_stride2(x as [64,64,320], sr_w) + sr_b   -> [1024, 320]
  x_ = LayerNorm(x_) * ln_g + ln_b
  kv = x_ @ kv_w + kv_b -> k, v  [1024, 320] each
  out = softmax(q k^T / 8) v  per head -> [4096, 320]
  out = out @ proj_w + proj_b

Sharding: pure data parallelism, batch i -> core i (B == n_cores == 8).
No collectives.

Per-core layout strategy ("T-layout", channels on partitions, tokens on
free dim):
  - x is loaded naturally and PE-transposed into xP: a patch-major view
    x^T[c, token] with tokens grouped by conv-patch offset block
    b = di*2+dj, so both the q projection and the conv reduce to plain
    matmuls over xP.  The token permutation is undone by the final
    strided DMA of the proj output.
  - all matmuls run in float32r (fp32 storage, 1 cyc/row on the PE).
  - biases are folded in via an appended ones-row (K+1 trick); LayerNorm
    gamma/beta are folded into kv_w/kv_b on the host.
  - the serial head (x ingest -> conv -> LN -> x_n^T -> k/v) is emitted
    window-by-window so the kv chain completes as early as possible; the
    q projection is interleaved into the attention loop, where ACT (exp)
    is the bottleneck and the PE has slack.
  - attention: scores are computed transposed (s^T[m, n]) so softmax's
    sum over keys m is done by the PE itself: v is augmented with a ones
    column, making row 64 of the PV accumulator the softmax denominator.
    exp() runs on ACT fused with the 1/8 scale, psum->sbuf.
    Division by sumexp happens after PV: reciprocal (DVE) is partition-
    broadcast by the otherwise-idle GPSIMD engine and applied with one
    DVE multiply.  The division chain of pair i is emitted after pair
    i+1's matmuls so the in-order PE stream never stalls on it.
"""

import numpy as np

import concourse.bass as bass
import concourse.mybir as mybir
from concourse import bacc
from concourse.tile import TileContext

F32 = mybir.dt.float32
F32R = mybir.dt.float32r
AF = mybir.ActivationFunctionType
ALU = mybir.AluOpType
AX = mybir.AxisListType

B, N, C = 8, 4096, 320
HEAD, HD, SR = 5, 64, 2
NKV = 1024  # (64/2)*(64/2)
SCALE = HD ** -0.5
EPS = 1e-5
N_CORES = 8

# channel chunking of C=320 into partitions: 128 + 128 + 64 (+1 ones row)
CK = [(0, 128), (128, 128), (256, 64)]


def _csz(k, aug):
    return CK[k][1] + (1 if (aug and k == 2) else 0)


def build_nc(repeat=1, loop_part="all", div_mode="pe", ablate=()):
    nc = bacc.Bacc()

    x = nc.declare_dram_parameter("x", [N, C], F32, isOutput=False)
    qw = nc.declare_dram_parameter("qw", [C + 1, C], F32, isOutput=False)
    srw = nc.declare_dram_parameter("srw", [4 * C + 1, C], F32, isOutput=False)
    kw = nc.declare_dram_parameter("kw", [C + 1, C], F32, isOutput=False)
    vw = nc.declare_dram_parameter("vw", [C + 1, C], F32, isOutput=False)
    pw = nc.declare_dram_parameter("pw", [C + 1, C], F32, isOutput=False)
    ident = nc.declare_dram_parameter("ident", [128, 128], F32, isOutput=False)
    ones_c = nc.declare_dram_parameter("ones_c", [1, N], F32, isOutput=False)
    ones2 = nc.declare_dram_parameter("ones2", [128, 64], F32, isOutput=False)
    e5 = nc.declare_dram_parameter("e5", [HEAD, C], F32, isOutput=False)
    out = nc.declare_dram_parameter("out", [N, C], F32, isOutput=True)

    import contextlib

    with TileContext(nc) as tc:
        with (
            nc.allow_low_precision(reason="float32r tiles are full fp32 storage"),
            tc.tile_pool(name="const", bufs=1) as cpool,
            tc.tile_pool(name="main", bufs=1) as mpool,
            tc.tile_pool(name="ps", space="PSUM", bufs=1) as pspool,
            tc.For_i(0, repeat, 1)
            if (repeat > 1 and loop_part == "all")
            else contextlib.nullcontext(),
        ):
            # ---- tile allocations ----------------------------------------
            id_sb = cpool.tile([128, 128], F32R)
            qw_sb = [
                cpool.tile([_csz(k, True), C], F32R, name=f"qw_sb{k}") for k in range(3)
            ]
            kw_sb = [
                cpool.tile([_csz(k, True), C], F32R, name=f"kw_sb{k}") for k in range(3)
            ]
            vw_sb = [
                cpool.tile([_csz(k, True), C], F32R, name=f"vw_sb{k}") for k in range(3)
            ]
            pw_sb = [
                cpool.tile([_csz(k, True), C], F32R, name=f"pw_sb{k}") for k in range(3)
            ]
            srw_sb = [
                [
                    cpool.tile([_csz(k, b == 0), C], F32R, name=f"srw_sb{b}{k}")
                    for k in range(3)
                ]
                for b in range(4)
            ]
            eps_sb = cpool.tile([128, 1], F32)
            e5_sb = cpool.tile([HEAD, C], F32R)

            xP = [
                [
                    mpool.tile([_csz(k, True), NKV], F32R, name=f"xP{b}{k}")
                    for k in range(3)
                ]
                for b in range(4)
            ]
            xnT = [
                mpool.tile([_csz(k, True), NKV], F32R, name=f"xnT{k}") for k in range(3)
            ]
            kT = [
                mpool.tile([_csz(k, False), NKV], F32R, name=f"kT{k}") for k in range(3)
            ]
            # v' [1024, 5*65] stored as [128, 8*325]; col t*325 + h*65 + d,
            # d==64 is the ones column (softmax denominator trick)
            v_all = mpool.tile([128, 8 * 325], F32R)

            # ============ head: x ingest + conv + LayerNorm + x_n^T =======
            # software-pipelined by one window: while window g is being
            # transposed/scattered, window g-1 runs conv+LN and window g-2
            # runs its v projection, so no engine waits on the serial chain.
            with (
                tc.For_i(0, repeat, 1)
                if (repeat > 1 and loop_part == "head")
                else contextlib.nullcontext(),
                tc.tile_pool(name="px", bufs=1) as xpool,
            ):

                def emit_window_ingest(g):
                    xa = []
                    for ii in range(4):
                        i = g * 4 + ii
                        t = xpool.tile([128, C], F32R, name="xa", tag="xa", bufs=6)
                        nc.sync.dma_start(
                            out=t[:], in_=x[i * 128 : (i + 1) * 128, :].bitcast(F32R)
                        )
                        xa.append(t)
                    for k in range(3):
                        csz = _csz(k, False)
                        psA = pspool.tile(
                            [csz, 512], F32R, name="psA", tag="ps512", bufs=4
                        )
                        for ii in range(4):
                            nc.tensor.transpose(
                                psA[:, ii * 128 : (ii + 1) * 128],
                                xa[ii][:, CK[k][0] : CK[k][0] + csz],
                                id_sb[:],
                            )
                        # scatter the 512-token window into the 4 patch blocks
                        src = psA.rearrange("p (i a j c) -> p i a j c", i=4, a=2, j=32)
                        for b in range(4):
                            di, dj = b // 2, b % 2
                            dst = xP[b][k][0:csz, g * 128 : (g + 1) * 128].rearrange(
                                "p (i j) -> p i j", i=4
                            )
                            if g >= 4 or b % 2 == 0:
                                nc.vector.tensor_copy(dst, src[:, :, di, :, dj])
                            else:
                                nc.scalar.copy(dst, src[:, :, di, :, dj])

                def emit_conv_ln(t8):
                    psC = pspool.tile([128, C], F32, name="psC", tag="ps320", bufs=2)
                    idx = 0
                    for b in range(4):
                        for k in range(3):
                            kp = _csz(k, b == 0)
                            nc.tensor.matmul(
                                psC[:],
                                xP[b][k][0:kp, t8 * 128 : (t8 + 1) * 128],
                                srw_sb[b][k][0:kp, :],
                                start=(idx == 0),
                                stop=(idx == 11),
                            )
                            idx += 1
                    sum_t = xpool.tile([128, 1], F32, name="sum_t", tag="st1", bufs=4)
                    negmu = xpool.tile([128, 1], F32, name="negmu", tag="st2", bufs=4)
                    vsum = xpool.tile([128, 1], F32, name="vsum", tag="st3", bufs=4)
                    sd = xpool.tile([128, 1], F32, name="sd", tag="st4", bufs=4)
                    inv = xpool.tile([128, 1], F32, name="inv", tag="st5", bufs=4)
                    scr = xpool.tile([128, C], F32, name="scr", tag="scr", bufs=3)
                    scr2 = xpool.tile([128, C], F32, name="scr2", tag="scr2", bufs=2)
                    xn = xpool.tile([128, C], F32R, name="xn", tag="xn", bufs=3)

                    nc.vector.tensor_reduce(sum_t[:], psC[:], axis=AX.X, op=ALU.add)
                    nc.vector.tensor_scalar_mul(negmu[:], sum_t[:], -1.0 / C)
                    # centered = x - mu  (SBUF), then vsum = sum(centered^2)
                    nc.vector.tensor_scalar(
                        out=scr[:], in0=psC[:], scalar1=negmu[:], scalar2=None, op0=ALU.add
                    )
                    nc.vector.scalar_tensor_tensor(
                        out=scr2[:],
                        in0=scr[:],
                        scalar=0.0,
                        in1=scr[:],
                        op0=ALU.add,
                        op1=ALU.mult,
                        accum_out=vsum[:],
                    )
                    nc.scalar.activation(
                        sd[:], vsum[:], AF.Sqrt, scale=1.0 / C, bias=eps_sb[:]
                    )
                    nc.vector.reciprocal(inv[:], sd[:])
                    nc.vector.tensor_scalar_mul(xn[:], scr[:], inv[:])
                    # transpose x_n -> xnT
                    for k in range(3):
                        csz = _csz(k, False)
                        psD = pspool.tile(
                            [csz, 128], F32R, name="psD", tag="ps512", bufs=4
                        )
                        nc.tensor.transpose(
                            psD[:], xn[:, CK[k][0] : CK[k][0] + csz], id_sb[:]
                        )
                        nc.scalar.copy(xnT[k][0:csz, t8 * 128 : (t8 + 1) * 128], psD[:])

                def emit_v(t8):
                    psF = pspool.tile([128, C], F32, name="psF", tag="ps320", bufs=2)
                    for k in range(3):
                        kp = _csz(k, True)
                        nc.tensor.matmul(
                            psF[:],
                            xnT[k][0:kp, t8 * 128 : (t8 + 1) * 128],
                            vw_sb[k][0:kp, :],
                            start=(k == 0),
                            stop=(k == 2),
                        )
                    dst = v_all[:, t8 * 325 : (t8 + 1) * 325].rearrange(
                        "p (h e) -> p h e", h=5
                    )[:, :, 0:64]
                    nc.scalar.copy(dst, psF.rearrange("p (h d) -> p h d", h=5))

                def emit_kT(nn):
                    for m in range(3):
                        msz = _csz(m, False)
                        psE = pspool.tile(
                            [msz, 512], F32, name="psE", tag="ps512", bufs=4
                        )
                        for k in range(3):
                            kp = _csz(k, True)
                            nc.tensor.matmul(
                                psE[:],
                                kw_sb[k][:, CK[m][0] : CK[m][0] + msz],
                                xnT[k][0:kp, nn * 512 : (nn + 1) * 512],
                                start=(k == 0),
                                stop=(k == 2),
                            )
                        nc.scalar.copy(kT[m][:, nn * 512 : (nn + 1) * 512], psE[:])

                # priority order: identity + first x window, then conv
                # weights, then the pipelined window loop
                nc.sync.dma_start(out=id_sb[:], in_=ident[:].bitcast(F32R))
                nc.vector.memset(eps_sb[:], EPS)
                nc.sync.dma_start(out=e5_sb[:], in_=e5[:].bitcast(F32R))
                emit_window_ingest(0)
                for b in range(4):
                    base = 0 if b == 0 else 321 + (b - 1) * C
                    for k in range(3):
                        p = _csz(k, b == 0)
                        nc.sync.dma_start(
                            out=srw_sb[b][k][:],
                            in_=srw[base + CK[k][0] : base + CK[k][0] + p, :].bitcast(F32R),
                        )
                for b in range(4):
                    nc.sync.dma_start(
                        out=xP[b][2][64:65, :], in_=ones_c[:, 0:NKV].bitcast(F32R)
                    )
                for k in range(3):
                    r0, p = CK[k][0], _csz(k, True)
                    nc.sync.dma_start(out=vw_sb[k][:], in_=vw[r0 : r0 + p, :].bitcast(F32R))
                    nc.sync.dma_start(out=kw_sb[k][:], in_=kw[r0 : r0 + p, :].bitcast(F32R))
                nc.sync.dma_start(
                    out=xnT[2][64:65, :], in_=ones_c[:, 0:NKV].bitcast(F32R)
                )
                nc.sync.dma_start(
                    out=v_all.rearrange("p (t h e) -> p t h e", t=8, h=5)[:, :, :, 64],
                    in_=ones2[:, 0:40].rearrange("p (t h) -> p t h", t=8).bitcast(F32R),
                )

                for g in range(1, 8):
                    emit_window_ingest(g)
                    emit_conv_ln(g - 1)
                    if g >= 2:
                        emit_v(g - 2)
                    if g == 5:
                        emit_kT(0)
                    if g == 2:
                        for k in range(3):
                            r0, p = CK[k][0], _csz(k, True)
                            nc.sync.dma_start(
                                out=qw_sb[k][:], in_=qw[r0 : r0 + p, :].bitcast(F32R)
                            )
                            nc.sync.dma_start(
                                out=pw_sb[k][:], in_=pw[r0 : r0 + p, :].bitcast(F32R)
                            )
                emit_conv_ln(7)
                emit_v(6)
                emit_v(7)
                emit_kT(1)

            # ================= attention + q-proj + out-proj ==============
            with (
                tc.For_i(0, repeat, 1)
                if (repeat > 1 and loop_part == "att")
                else contextlib.nullcontext(),
                tc.tile_pool(name="att", bufs=1) as apool,
            ):
                attnT = [
                    apool.tile([_csz(k, True), N], F32R, name=f"attnT{k}")
                    for k in range(3)
                ]
                nc.sync.dma_start(
                    out=attnT[2][64:65, :], in_=ones_c[:, 0:N].bitcast(F32R)
                )

                out_r = out.rearrange("(i a j c) d -> i a j c d", a=2, j=32, c=2)

                def emit_q(n):
                    # q^T for 512-token column chunk n -> rotating tiles
                    b, nn = n // 2, n % 2
                    tiles = []
                    for m in range(3):
                        msz = _csz(m, False)
                        psB = pspool.tile(
                            [msz, 512], F32, name="psB", tag="ps320", bufs=2
                        )
                        for k in range(3):
                            kp = _csz(k, True)
                            nc.tensor.matmul(
                                psB[:],
                                qw_sb[k][:, CK[m][0] : CK[m][0] + msz],
                                xP[b][k][0:kp, nn * 512 : (nn + 1) * 512],
                                start=(k == 0),
                                stop=(k == 2),
                            )
                        qtn = apool.tile(
                            [msz, 512], F32R, name=f"qTn{m}", tag=f"qtn{m}", bufs=2
                        )
                        nc.vector.tensor_copy(qtn[:], psB[:])
                        tiles.append(qtn)
                    return tiles

                se_all = apool.tile([1, HEAD * 512], F32)
                se5s = {}

                def emit_division_s1(n, h, psO):
                    # drain psO: unnormalized out -> attnT, sumexp -> se1 row
                    hc, hr = h // 2, (h % 2) * 64
                    ns = slice(n * 512, (n + 1) * 512)
                    nc.vector.tensor_copy(attnT[hc][hr : hr + 64, ns], psO[0:64, :])
                    nc.vector.tensor_copy(
                        se_all[0:1, h * 512 : (h + 1) * 512], psO[64:65, :]
                    )
                    if h == HEAD - 1:
                        # scatter the 5 sumexp rows onto 5 partitions via DMA
                        se5 = apool.tile([HEAD, 512], F32, name="se5", tag="se5", bufs=2)
                        nc.sync.dma_start(out=se5[:], in_=se_all[0:1, :])
                        se5s[n] = se5
                    return n

                def emit_division_s2(n):
                    # batched per n-chunk: one reciprocal, K=5 head-select
                    # broadcast matmul, 3 in-place multiplies
                    if "div" in ablate:
                        return
                    ns = slice(n * 512, (n + 1) * 512)
                    rec5 = apool.tile([HEAD, 512], F32R, name="rec5", tag="rec5", bufs=2)
                    nc.vector.reciprocal(rec5[:], se5s[n][:])
                    for k in range(3):
                        csz = _csz(k, False)
                        psR = pspool.tile(
                            [csz, 512], F32, name="psR", tag="ps320", bufs=2
                        )
                        nc.tensor.matmul(
                            psR[:],
                            e5_sb[:, CK[k][0] : CK[k][0] + csz],
                            rec5[:],
                            start=True,
                            stop=True,
                        )
                        nc.vector.tensor_tensor(
                            out=attnT[k][0:csz, ns],
                            in0=attnT[k][0:csz, ns],
                            in1=psR[:],
                            op=ALU.mult,
                        )

                proj_queue = []

                def emit_proj_unit():
                    if not proj_queue:
                        return
                    t = proj_queue.pop(0)
                    psP = pspool.tile([128, C], F32, name="psP", tag="ps320", bufs=2)
                    for k in range(3):
                        kp = _csz(k, True)
                        nc.tensor.matmul(
                            psP[:],
                            attnT[k][0:kp, t * 128 : (t + 1) * 128],
                            pw_sb[k][0:kp, :],
                            start=(k == 0),
                            stop=(k == 2),
                        )
                    o_sb = apool.tile([128, C], F32, name="o_sb", tag="o_sb", bufs=4)
                    nc.vector.tensor_copy(o_sb[:], psP[:])
                    b, i0 = t // 8, (t % 8) * 4
                    di, dj = b // 2, b % 2
                    nc.sync.dma_start(out=out_r[i0 : i0 + 4, di, :, dj, :], in_=o_sb[:])

                # q for chunk 0 before attention starts; later chunks are
                # interleaved (the PE has slack in the ACT-bound pairs)
                qcur = emit_q(0)

                div_s1 = None  # pair awaiting division stage 1
                div_s2_q = []  # stage-1-done divisions awaiting the multiply
                for n in range(8):  # 512-token column chunks (xP order)
                    qnext = None
                    for h in range(HEAD):
                        hc, hr = h // 2, (h % 2) * 64
                        psO = pspool.tile([65, 512], F32, name="psO", tag="psacc", bufs=2)
                        p_tiles = []
                        # software-pipelined: scores(mc+1) issued before PV(mc)
                        for mc in range(8):
                            psS = pspool.tile(
                                [128, 512], F32, name="psS", tag="ps512", bufs=4
                            )
                            nc.tensor.matmul(
                                psS[:],
                                kT[hc][hr : hr + 64, mc * 128 : (mc + 1) * 128],
                                qcur[hc][hr : hr + 64, :],
                                start=True,
                                stop=True,
                            )
                            p_t = apool.tile(
                                [128, 512], F32R, name="p_t", tag="p_t", bufs=5
                            )
                            nc.scalar.activation(p_t[:], psS[:], AF.Exp, scale=SCALE)
                            p_tiles.append(p_t)
                            if mc == 2 and div_s1 is not None:
                                nfin = emit_division_s1(*div_s1)
                                if div_s1[1] == HEAD - 1:
                                    div_s2_q.append(nfin)
                                div_s1 = None
                            if mc == 5:
                                if div_s2_q:
                                    ndone = div_s2_q.pop(0)
                                    emit_division_s2(ndone)
                                    proj_queue.extend(
                                        range(ndone * 4, ndone * 4 + 4)
                                    )
                                if "proj" not in ablate:
                                    emit_proj_unit()
                            if mc >= 2:
                                nc.tensor.matmul(
                                    psO[:],
                                    v_all[:, (mc - 2) * 325 + h * 65 : (mc - 2) * 325 + (h + 1) * 65],
                                    p_tiles[mc - 2][:],
                                    start=(mc - 2 == 0),
                                    stop=False,
                                    skip_group_check=True,
                                )
                        for mct in (6, 7):
                            nc.tensor.matmul(
                                psO[:],
                                v_all[:, mct * 325 + h * 65 : mct * 325 + (h + 1) * 65],
                                p_tiles[mct][:],
                                start=False,
                                stop=(mct == 7),
                                skip_group_check=True,
                            )
                        div_s1 = (n, h, psO)
                        # next chunk's q after the first pair of this chunk
                        if h == 0 and n < 7:
                            qnext = emit_q(n + 1) if "q" not in ablate else qcur
                    qcur = qnext
                nlast = emit_division_s1(*div_s1)
                div_s2_q.append(nlast)
                while div_s2_q:
                    ndone = div_s2_q.pop(0)
                    emit_division_s2(ndone)
                    proj_queue.extend(range(ndone * 4, ndone * 4 + 4))
                if "proj" not in ablate:
                    while proj_queue:
                        emit_proj_unit()

    nc.compile()
    return nc


_CACHE = {}


def _prep_inputs(inputs):
    x = np.ascontiguousarray(np.asarray(inputs["x"], dtype=np.float32))
    q_w = np.asarray(inputs["q_w"], np.float32)
    q_b = np.asarray(inputs["q_b"], np.float32)
    kv_w = np.asarray(inputs["kv_w"], np.float32)
    kv_b = np.asarray(inputs["kv_b"], np.float32)
    sr_w = np.asarray(inputs["sr_w"], np.float32)
    sr_b = np.asarray(inputs["sr_b"], np.float32)
    ln_g = np.asarray(inputs["ln_g"], np.float32)
    ln_b = np.asarray(inputs["ln_b"], np.float32)
    proj_w = np.asarray(inputs["proj_w"], np.float32)
    proj_b = np.asarray(inputs["proj_b"], np.float32)

    qw = np.concatenate([q_w, q_b[None, :]], axis=0)  # [321, 320]
    blocks = sr_w.reshape(4, C, C)  # HWIO -> (di*2+dj, ci, co)
    srw = np.concatenate(
        [blocks[0], sr_b[None, :], blocks[1], blocks[2], blocks[3]], axis=0
    )  # [1281, 320]
    kv_w_eff = ln_g[:, None] * kv_w
    kv_bias = ln_b @ kv_w + kv_b
    kw = np.concatenate([kv_w_eff[:, :C], kv_bias[None, :C]], axis=0)
    vw = np.concatenate([kv_w_eff[:, C:], kv_bias[None, C:]], axis=0)
    pw = np.concatenate([proj_w, proj_b[None, :]], axis=0)

    e5 = np.zeros((HEAD, C), np.float32)
    for h in range(HEAD):
        e5[h, h * HD : (h + 1) * HD] = 1.0
    consts = {
        "e5": e5,
        "qw": np.ascontiguousarray(qw),
        "srw": np.ascontiguousarray(srw),
        "kw": np.ascontiguousarray(kw),
        "vw": np.ascontiguousarray(vw),
        "pw": np.ascontiguousarray(pw),
        "ident": np.eye(128, dtype=np.float32),
        "ones_c": np.ones((1, N), np.float32),
        "ones2": np.ones((128, 64), np.float32),
    }
    return x, consts


def kernel(**inputs) -> np.ndarray:
    H = int(np.asarray(inputs["H"]))
    W = int(np.asarray(inputs["W"]))
    assert H == 64 and W == 64, (H, W)
    x, consts = _prep_inputs(inputs)
    assert x.shape == (B, N, C)

    if "nc" not in _CACHE:
        _CACHE["nc"] = build_nc()
    nc = _CACHE["nc"]

    from concourse.bass_utils import run_bass_kernel_spmd

    in_maps = [{"x": x[i], **consts} for i in range(N_CORES)]
    res = run_bass_kernel_spmd(nc, in_maps, core_ids=list(range(N_CORES)))
    out = np.stack([res.results[i]["out"] for i in range(N_CORES)], axis=0)
    return out.astype(np.float32)
